# revision 12
# baseline (speedup 1.0000x reference)
"""Trainium2 Bass kernel for nn_ClassWiseResponseMemory.

Reference semantics (per sample i, in batch order):
    c = counts[t_i];  is_init = c <= 0  (START=0, UPDATE_INTERVAL=1)
    new = r_i                         if is_init
        = 0.9 * mem[t_i] + 0.1 * r_i  otherwise
    mem[t_i] = new; counts[t_i] += 1; out[i] = new

Chains only couple samples of the SAME class and every feature is
independent, so:
  1. (host, integer logic) stably sort samples by class; derive the
     per-position init flag s_t (state reset points).  Samples of one
     class form a contiguous segment in sorted order.
  2. (device) first-order linear recurrence along the sorted axis with
     the DVE scan: state = a_t * state + b_t * r_t, a_t in {0, 1-m},
     b_t in {1, m}.  Features on SBUF partitions, sorted-sample axis on
     the free dim.
  3. (host) scatter sorted results back to batch order.

Sharding: features split 2048 -> 8 x 256 across cores.  The two
128-row feature groups of a core are CONCATENATED along the free dim
([128, 2T]): column 0 of the sorted stream always has s=1 (a=0), so one
chunk-chained scan covers both halves with no special casing.

Device pipeline (per core):
  * data plane is fp16 both ways: HBM bytes per core drop from the fp32
    baseline's 8.9MB to 4.1MB (DMA roofline ~12us at ~358 GB/s).
  * init flags go up as sx [4, Tp] fp16 (rows s, s, 1, 1; 32KB).  The
    PE broadcasts/affines them into the a-coefficient plane directly in
    PSUM: a = -(c_hi+c_lo)*s + (c_hi+c_lo), with 0.9 split into two
    fp16 constants so a matches fp32 (1-m) to ~3e-8 and is exactly 0 at
    init positions.  The DVE scan reads data0 straight from PSUM.
  * ScalarE's only compute is b = 1 - a (PSUM -> SBUF fp32, exact 1.0
    at init); its HWDGE ring then carries the output stores.
  * premultiply r *= b runs on GpSimd; chained scans on DVE; loads on
    the sync HWDGE ring; chunks get finer toward the end of each half
    so the last premult->scan->store tail is short.
"""

import os
from contextlib import ExitStack

import numpy as np

N_CORES = 8
P = 128
MOMENTUM = 0.1
START = 0
UPDATE_INTERVAL = 1
MM_BLK = 512      # one PSUM bank of fp32
PSUM_COLS = 4096  # PSUM capacity per partition in fp32
# per-half chunk widths for load/premult/scan/store, fine at both ends
# (short lead-in, short tail); env-overridable for HW A/B sweeps
CHUNKS = tuple(
    int(x) for x in os.environ.get("CWRM_CHUNKS", "512,1024,2048,512").split(",")
)

# fp32-exact constants matching the reference's float32 arithmetic
_AM = float(np.float32(1.0) - np.float32(MOMENTUM))  # (1 - momentum) in fp32
_M = float(np.float32(MOMENTUM))
# two-term fp16 split of (1 - m): c_hi + c_lo == _AM to ~3e-8
_C_HI = float(np.float16(_AM))
_C_LO = float(np.float16(np.float32(_AM) - np.float32(_C_HI)))

_compiled_cache: dict = {}


def _scan_chunks(T: int):
    """Per-half chunk bounds from the CHUNKS pattern (tiled to cover T;
    general-T fallback is plain 2048 steps)."""
    if T == sum(CHUNKS):
        pieces = list(CHUNKS)
    else:
        pieces = []
        rem = T
        while rem > 0:
            take = min(2048, rem)
            pieces.append(take)
            rem -= take
    bounds = []
    lo = 0
    for w in pieces:
        bounds.append((lo, lo + w))
        lo += w
    assert lo == T
    return bounds


def _build_nc(T: int):
    """Per-core program.  Inputs: r [P, 2T] fp16 (feature-sliced,
    class-sorted, transposed, two feature groups concatenated along the
    free dim), sx [4, Tp] fp16 (rows s, s, 1, 1).  Output: o [P, 2T]."""
    import concourse.bacc as bacc
    import concourse.mybir as mybir
    import concourse.tile as tile
    from concourse.tile_rust import add_dep_helper

    Tp = (T + MM_BLK - 1) // MM_BLK * MM_BLK
    L = 2 * T
    f16, f32 = mybir.dt.float16, mybir.dt.float32
    psum_direct = Tp <= PSUM_COLS

    nc = bacc.Bacc("TRN2", target_bir_lowering=False, debug=False)
    r_in = nc.dram_tensor("r", [P, L], f16, kind="ExternalInput").ap()
    s_in = nc.dram_tensor("sx", [4, Tp], f16, kind="ExternalInput").ap()
    w_in = nc.dram_tensor("wa", [4, P], f16, kind="ExternalInput").ap()
    o_out = nc.dram_tensor("o", [P, L], f16, kind="ExternalOutput").ap()

    def chain(insts):
        for i_prev, i_next in zip(insts, insts[1:]):
            add_dep_helper(i_next.ins, i_prev.ins, False, "order")

    with tile.TileContext(nc) as tc:
        with ExitStack() as ctx:
            pool = ctx.enter_context(tc.tile_pool(name="sbuf", bufs=1))
            ppool = ctx.enter_context(
                tc.tile_pool(name="psum", bufs=1, space="PSUM")
            )

            sx_t = pool.tile([4, Tp], f16, tag="sx")
            # lhsT weights for the coefficient matmul (host-sent 1KB):
            # a = -(c_hi*s + c_lo*s) + (c_hi + c_lo)
            wa_t = pool.tile([4, P], f16, tag="wa")
            ld_wa = nc.sync.dma_start(wa_t[:], w_in[:])
            ld_sx = nc.sync.dma_start(sx_t[:], s_in[:])
            chain([ld_wa, ld_sx])

            # r chunks stream in on the sync HWDGE ring
            r_t = pool.tile([P, L], f16, tag="r")
            half_bounds = _scan_chunks(T)
            bounds = [
                (base + lo, base + hi)
                for base in (0, T)
                for (lo, hi) in half_bounds
            ]
            loads = [ld_sx]
            for lo, hi in bounds:
                loads.append(nc.sync.dma_start(r_t[:, lo:hi], r_in[:, lo:hi]))
            chain(loads)

            # PE: a-plane into PSUM, one 512-col bank at a time
            if psum_direct:
                a_ps = ppool.tile([P, Tp], f32, tag="aps")
                for h in range(0, Tp, MM_BLK):
                    nc.tensor.matmul(
                        a_ps[:, h : h + MM_BLK],
                        wa_t[:],
                        sx_t[:, h : h + MM_BLK],
                        start=True,
                        stop=True,
                    )
                a_src = a_ps

                # ScalarE: b = 1 - a (exact at init), PSUM -> SBUF fp32,
                # blocked like the chunks so the first premult's b slice
                # is ready as early as possible
                b_t = pool.tile([P, Tp], f32, tag="b")
                b_bounds = list(_scan_chunks(T))
                if Tp > T:
                    b_bounds.append((T, Tp))
                for j, w_hi in b_bounds:
                    nc.scalar.activation(
                        b_t[:, j:w_hi],
                        a_ps[:, j:w_hi],
                        mybir.ActivationFunctionType.Copy,
                        scale=-1.0,
                        bias=1.0,
                    )
            else:
                # general path (T too big for PSUM-resident a): rotate
                # two PSUM tiles, copy a and b out to SBUF
                a_sb = pool.tile([P, Tp], f32, tag="a")
                b_t = pool.tile([P, Tp], f32, tag="b")
                ps = [
                    ppool.tile([P, 1024], f32, tag=f"ps{i}", name=f"ps{i}")
                    for i in range(2)
                ]
                for jj, j in enumerate(range(0, Tp, 1024)):
                    w = min(1024, Tp - j)
                    p_t = ps[jj % 2]
                    for h in range(j, j + w, MM_BLK):
                        nc.tensor.matmul(
                            p_t[:, h - j : h - j + MM_BLK],
                            wa_t[:],
                            sx_t[:, h : h + MM_BLK],
                            start=True,
                            stop=True,
                        )
                    nc.scalar.activation(
                        b_t[:, j : j + w],
                        p_t[:, :w],
                        mybir.ActivationFunctionType.Copy,
                        scale=-1.0,
                        bias=1.0,
                    )
                    nc.scalar.activation(
                        a_sb[:, j : j + w],
                        p_t[:, :w],
                        mybir.ActivationFunctionType.Copy,
                        scale=1.0,
                        bias=0.0,
                    )
                a_src = a_sb

            # premult (GpSimd, in place) -> chained scan (DVE) -> store
            # (scalar HWDGE ring), chunk by chunk
            o_t = pool.tile([P, L], f16, tag="o")
            premults, scans, stores = [], [], []
            for k, (lo, hi) in enumerate(bounds):
                alo = lo % T
                ahi = alo + (hi - lo)
                premults.append(
                    nc.gpsimd.tensor_tensor(
                        out=r_t[:, lo:hi],
                        in0=r_t[:, lo:hi],
                        in1=b_t[:, alo:ahi],
                        op=mybir.AluOpType.mult,
                    )
                )
                scans.append(
                    nc.vector.tensor_tensor_scan(
                        out=o_t[:, lo:hi],
                        data0=a_src[:, alo:ahi],
                        data1=r_t[:, lo:hi],
                        initial=0.0 if k == 0 else o_t[:, lo - 1 : lo],
                        op0=mybir.AluOpType.mult,
                        op1=mybir.AluOpType.add,
                    )
                )
                st_eng = nc.scalar if k % 2 == 0 else nc.sync
                stores.append(st_eng.dma_start(o_out[:, lo:hi], o_t[:, lo:hi]))
            chain(premults)
            # stores alternate between the two HWDGE rings; keep FIFO
            # order within each ring only (the sync ring's stores sit
            # after all loads, so they can never stall a load)
            chain(stores[0::2])
            chain(loads + stores[1::2])
    nc.compile()
    return nc


def _preprocess(targets: np.ndarray, counts: np.ndarray):
    """Integer-only index prep from targets/counts.

    Returns (src_idx, is_mem, s_flags, out_pos):
      src_idx[t]: column t of the device input takes responses[src_idx[t]]
                  (or memory[src_idx[t]] where is_mem[t])
      s_flags[t]: 1 where the scan state must reset to the column value
      out_pos:    orig sample index per column, -1 for prepended mem columns
    """
    B = targets.shape[0]
    perm = np.argsort(targets, kind="stable").astype(np.int64)
    tsort = targets[perm]
    start = np.ones(B, dtype=bool)
    if B > 1:
        start[1:] = tsort[1:] != tsort[:-1]
    seg_id = np.cumsum(start) - 1
    first_pos = np.zeros(seg_id[-1] + 1 if B else 0, dtype=np.int64)
    first_pos[seg_id[start]] = np.nonzero(start)[0]
    occ = np.arange(B, dtype=np.int64) - first_pos[seg_id]
    c = counts[tsort].astype(np.int64) + occ
    # UPDATE_INTERVAL == 1 -> do_update always true
    assert UPDATE_INTERVAL == 1
    is_init = c <= START

    need_pre = start & ~is_init  # first occurrence blends with memory[class]
    if not need_pre.any():
        return (
            perm,
            np.zeros(B, dtype=bool),
            is_init.astype(np.uint8),
            perm,
        )

    # general path: prepend a memory[class] column before such segments
    n_pre = int(need_pre.sum())
    T = B + n_pre
    src_idx = np.empty(T, dtype=np.int64)
    is_mem = np.zeros(T, dtype=bool)
    s_flags = np.empty(T, dtype=np.uint8)
    out_pos = np.empty(T, dtype=np.int64)
    ins_before = np.cumsum(need_pre) - need_pre  # prepends before position t
    pos = np.arange(B) + ins_before + need_pre  # final position of sample t
    pre_at = pos[need_pre] - 1
    src_idx[pos] = perm
    is_mem[pos] = False
    s_flags[pos] = is_init.astype(np.uint8)
    out_pos[pos] = perm
    src_idx[pre_at] = tsort[need_pre]
    is_mem[pre_at] = True
    s_flags[pre_at] = 1
    out_pos[pre_at] = -1
    return src_idx, is_mem, s_flags, out_pos


def kernel(responses, targets, memory, counts):
    from concourse.bass_utils import run_bass_kernel_spmd

    responses = np.ascontiguousarray(np.asarray(responses, dtype=np.float32))
    targets = np.asarray(targets, dtype=np.int32)
    memory = np.asarray(memory, dtype=np.float32)
    counts = np.asarray(counts, dtype=np.int32)

    B, F = responses.shape
    assert F % (N_CORES * P) == 0
    n_groups = F // (N_CORES * P)

    src_idx, is_mem, s_flags, out_pos = _preprocess(targets, counts)
    T = len(src_idx)
    Tp = (T + MM_BLK - 1) // MM_BLK * MM_BLK

    if T not in _compiled_cache:
        _compiled_cache[T] = _build_nc(T)
    nc = _compiled_cache[T]

    # assemble sorted (and possibly mem-extended) rows: [T, F] fp16
    if is_mem.any():
        rows = np.empty((T, F), dtype=np.float32)
        rows[~is_mem] = responses[src_idx[~is_mem]]
        rows[is_mem] = memory[src_idx[is_mem]]
        rows16 = rows.astype(np.float16)
    else:
        rows16 = responses.astype(np.float16)[src_idx]

    sx = np.ones((4, Tp), dtype=np.float16)
    sx[0, :T] = s_flags
    sx[1, :T] = s_flags
    wa = np.empty((4, P), dtype=np.float16)
    for i, v in enumerate((-_C_HI, -_C_LO, _C_HI, _C_LO)):
        wa[i, :] = v

    in_maps = []
    for k in range(N_CORES):
        d_core = np.empty((P, 2 * T), dtype=np.float16)
        for g in range(n_groups):
            f0 = k * n_groups * P + g * P
            d_core[:, g * T : (g + 1) * T] = rows16[:, f0 : f0 + P].T
        in_maps.append({"r": d_core, "sx": sx, "wa": wa})

    want_trace = bool(os.environ.get("CWRM_TRACE"))
    if not want_trace:
        # the trace path needs an axon NTFF hook this container may lack;
        # make sure a stray BASS_TRACE can't route us there
        os.environ["BASS_NEVER_TRACE"] = "1"
    res = run_bass_kernel_spmd(
        nc,
        in_maps,
        core_ids=list(range(N_CORES)),
        trace=want_trace,
    )
    global LAST_RESULTS
    LAST_RESULTS = res

    out = np.empty((B, F), dtype=np.float32)
    keep = out_pos >= 0
    kept_pos = out_pos[keep]
    for k in range(N_CORES):
        o_core = res.results[k]["o"]  # [P, 2T] fp16
        for g in range(n_groups):
            f0 = k * n_groups * P + g * P
            out[kept_pos, f0 : f0 + P] = (
                o_core[:, g * T : (g + 1) * T].T[keep].astype(np.float32)
            )
    return out


LAST_RESULTS = None
